# revision 18
# baseline (speedup 1.0000x reference)
"""Trainium2 Bass kernel for the GNN message-passing network.

Sharding: 16384 nodes split across 8 NeuronCores (2048 nodes/core).
Tables and weights are replicated; per-core index/selector tensors drive
dma_gather row gathers and selector-matmul segment sums (PSUM fp32).

Key optimizations:
- h0/h1 are exchanged and gathered in fp8e4 (scaled by 256/64), and the
  segment-sum selector matmuls run fp8 x fp8; the interpro table is
  gathered in fp8 (x16). Measured end-to-end rel err ~1.5e-2 (< 2e-2).
- The esm half of the final matmul (target-protein rows x W_out[:,D:])
  only depends on a gather, so it runs inside the two AllGather walls
  where the tensor engine would otherwise idle; partial outputs park in
  DRAM (bf16) and are added back in the final phase.
- cat matrices are transposed on-chip (SBUF->SBUF DMA transpose); the
  layer-1 update and final matmul run output-transposed with stationary
  weight tiles, so h2^T stays SBUF-resident. Output is [L, nodes] per
  core; the host transposes.
"""
import numpy as np
import ml_dtypes

import concourse.bacc as bacc
import concourse.mybir as mybir
import concourse.tile as tile
from concourse import bass_utils

BF16 = ml_dtypes.bfloat16
FP8 = ml_dtypes.float8_e4m3

# Problem shapes (fixed).
N = 16384
E = 262144
T = 327680
P = 20000
IP = 30000
D_ESM = 1280
D = 1024
L = 5000
LP = 5120                 # L padded to 128 multiple
NLS = LP // 128           # 40 L-sub tiles
G = 2
NCORES = 8
NS = N // NCORES          # 2048 nodes per core
NBLK = NS // 128          # 16 dst blocks per core
KE = D_ESM // 128         # 10 k-chunks for esm
KU = (2 * D) // 128       # 16 k-chunks for update matmul
S_IPW = 16.0              # interpro table fp8 scale
S_H = [256.0, 64.0]       # h0 / h1 fp8 scales

# Set to 0/2 to truncate the kernel for debugging (test.py uses this).
PHASES = 3
TRACE = False


def _wrap_idx(idx, total):
    """[128, total/16] int16: token i at (i%16, i//16), replicated x8 groups."""
    a = np.zeros(total, np.int16)
    a[: len(idx)] = idx.astype(np.int16)
    blk = a.reshape(total // 16, 16).T
    return np.tile(blk, (8, 1)).copy()


def _pack_stream(tok_idx_per_block, dcol_per_block, val_per_block, ch_per_block):
    """Build padded token stream + selector array for one core."""
    tot = sum(ch_per_block) * 128
    idx_s = np.zeros(tot, np.int64)
    pos_l = []
    col_l = []
    val_l = []
    base = 0
    for b in range(len(ch_per_block)):
        tok = tok_idx_per_block[b]
        n = len(tok)
        idx_s[base : base + n] = tok
        pos = base + np.arange(n)
        pos_l.append(pos)
        col_l.append(dcol_per_block[b])
        val_l.append(
            val_per_block[b] if val_per_block is not None else np.ones(n, np.float32)
        )
        base += ch_per_block[b] * 128
    pos = np.concatenate(pos_l) if pos_l else np.zeros(0, np.int64)
    col = np.concatenate(col_l).astype(np.int64) if col_l else np.zeros(0, np.int64)
    val = np.concatenate(val_l) if val_l else np.zeros(0, np.float32)
    return idx_s, pos, col, val


def _sel_array(pos, col, val, totc):
    """[128, totc*128] fp8 selector: S[pos%128, (pos//128)*128 + col] = val."""
    sel = np.zeros((128, totc * 128), np.float32)
    sel[pos % 128, (pos // 128) * 128 + col] = val
    return sel.astype(FP8)


def _pairs(c0, c1):
    """DoubleRow pairing measured slower on HW; emit singles."""
    return [(ci, 1) for ci in range(c0, c1)]


def _units(totc):
    """Split totc 128-token chunks into gather units of <=8 chunks."""
    out = []
    c0 = 0
    while c0 < totc:
        n = min(8, totc - c0)
        out.append((c0, n))
        c0 += n
    return out


def preprocess(inputs):
    """Host-side: shard, sort edges by dst, build index/selector tensors."""
    prot = np.asarray(inputs["protein_embedding"], np.float32)
    ipw = np.asarray(inputs["interpro_weight"], np.float32)
    W_esm = np.asarray(inputs["W_esm"], np.float32)
    b_esm = np.asarray(inputs["b_esm"], np.float32)
    bias1 = np.asarray(inputs["bias1"], np.float32)
    bias2 = np.asarray(inputs["bias2"], np.float32)
    w = np.asarray(inputs["w"], np.float32)
    W_upd = np.asarray(inputs["W_upd"], np.float32)
    b_upd = np.asarray(inputs["b_upd"], np.float32)
    W_out = np.asarray(inputs["W_out"], np.float32)
    b_out = np.asarray(inputs["b_out"], np.float32)
    self_w = np.asarray(inputs["self_w"], np.float32)
    ppi_w = np.asarray(inputs["ppi_w"], np.float32)
    node_in = np.asarray(inputs["inputs"], np.int64)
    ip_idx = np.asarray(inputs["interpro_idx"], np.int64)
    ip_off = np.asarray(inputs["interpro_off"], np.int64)
    src = np.asarray(inputs["src"], np.int64)
    dst = np.asarray(inputs["dst"], np.int64)
    target = np.asarray(inputs["target_id"], np.int64)

    ew = np.exp(w - w.max())
    sm = ew / ew.sum()

    bias_x1 = b_esm + bias1

    # --- edges: per (core, block) token lists sorted by dst ---
    order = np.argsort(dst, kind="stable")
    src_s, dst_s = src[order], dst[order]
    sw_s, pw_s = self_w[order], ppi_w[order]
    gblk = dst_s // 128
    blk_counts = np.bincount(gblk, minlength=N // 128)
    blk_starts = np.concatenate([[0], np.cumsum(blk_counts)])
    ch_e = np.zeros((NCORES, NBLK), np.int64)
    for c in range(NCORES):
        for b in range(NBLK):
            ch_e[c, b] = -(-blk_counts[c * NBLK + b] // 128)
    CH_E = ch_e.max(axis=0)
    TOTC_E = int(CH_E.sum())

    # --- bags ---
    bag_sizes = ip_off[1:] - ip_off[:-1]
    ch_b = np.zeros((NCORES, NBLK), np.int64)
    for c in range(NCORES):
        for b in range(NBLK):
            n0 = c * NS + b * 128
            cnt = int(ip_off[n0 + 128] - ip_off[n0])
            ch_b[c, b] = max(1, -(-cnt // 128))
    CH_B = ch_b.max(axis=0)
    TOTC_B = int(CH_B.sum())

    meta = dict(
        sm0=float(sm[0]),
        sm1=float(sm[1]),
        CH_E=[int(x) for x in CH_E],
        CH_B=[int(x) for x in CH_B],
        has_bias_x1=bool(np.any(bias_x1 != 0)),
        has_bias_x2=bool(np.any(bias2 != 0)),
        has_bias_upd=bool(np.any(b_upd != 0)),
        has_bias_out=bool(np.any(b_out != 0)),
    )

    W_esmT = np.ascontiguousarray(
        W_esm.T.reshape(KE, 128, D).transpose(1, 0, 2)
    ).astype(BF16)  # [128, KE, D]
    W_updT = np.ascontiguousarray(
        W_upd.transpose(0, 2, 1).reshape(G, KU, 128, D).transpose(0, 2, 1, 3)
    ).astype(BF16)  # [G, 128, KU, D]
    W_outP = np.zeros((18 * 128, LP), np.float32)
    W_outP[: D + D_ESM, :L] = W_out.T
    W_outT = np.ascontiguousarray(
        W_outP.reshape(18, 128, LP).transpose(1, 0, 2)
    ).astype(BF16)  # [128, 18, LP]
    # row-oriented biases (free-dim broadcast via ones-matmul)
    cbias = np.zeros((1, 128 + 3 * D), np.float32)
    cbias[0, :128] = 1.0
    cbias[0, 128 : 128 + D] = bias_x1
    cbias[0, 128 + D : 128 + 2 * D] = bias2
    cbias[0, 128 + 2 * D : 128 + 3 * D] = b_upd[0]
    # partition-oriented biases for transposed outputs (scalar bias add)
    cbT = np.zeros((128, 8 + NLS), np.float32)
    cbT[:, 0:8] = b_upd[1].reshape(8, 128).T
    cbT[:, 8 : 8 + NLS] = np.pad(b_out, (0, LP - L)).reshape(NLS, 128).T
    shared = dict(
        prot=prot.astype(BF16),
        ipw=(ipw * S_IPW).astype(FP8),
        W_esmT=W_esmT,
        W_updT=W_updT,
        W_outT=W_outT,
        cbias=cbias.astype(BF16),
        cbT=cbT.astype(np.float32),
    )

    in_maps = []
    for c in range(NCORES):
        esm_idx = _wrap_idx(node_in[c * NS : (c + 1) * NS], NS)
        tgt_idx = _wrap_idx(target[c * NS : (c + 1) * NS], NS)

        tokb, colb, valsb, valpb = [], [], [], []
        for b in range(NBLK):
            s0, s1 = blk_starts[c * NBLK + b], blk_starts[c * NBLK + b + 1]
            tokb.append(src_s[s0:s1])
            colb.append(dst_s[s0:s1] - (c * NS + b * 128))
            valsb.append(sw_s[s0:s1])
            valpb.append(pw_s[s0:s1])
        eidx, epos, ecol, esv = _pack_stream(tokb, colb, valsb, list(CH_E))
        _, _, _, epv = _pack_stream(tokb, colb, valpb, list(CH_E))
        sel_self = _sel_array(epos, ecol, esv, TOTC_E)
        sel_ppi = _sel_array(epos, ecol, epv, TOTC_E)

        tokb, colb = [], []
        for b in range(NBLK):
            n0 = c * NS + b * 128
            i0, i1 = int(ip_off[n0]), int(ip_off[n0 + 128])
            tokb.append(ip_idx[i0:i1])
            colb.append(
                np.repeat(
                    np.arange(128), bag_sizes[n0 : n0 + 128].astype(np.int64)
                )
            )
        bidx, bpos, bcol, bval = _pack_stream(tokb, colb, None, list(CH_B))
        sel_bag = _sel_array(bpos, bcol, bval, TOTC_B)

        m = dict(shared)
        m.update(
            esm_idx=esm_idx,
            tgt_idx=tgt_idx,
            e_idx=_wrap_idx(eidx, TOTC_E * 128),
            b_idx=_wrap_idx(bidx, TOTC_B * 128),
            sel_self=sel_self,
            sel_ppi=sel_ppi,
            sel_bag=sel_bag,
        )
        in_maps.append(m)
    return meta, in_maps


def build(meta):
    CH_E = meta["CH_E"]
    CH_B = meta["CH_B"]
    TOTC_E = sum(CH_E)
    TOTC_B = sum(CH_B)
    sm0, sm1 = meta["sm0"], meta["sm1"]
    bf = mybir.dt.bfloat16
    f32 = mybir.dt.float32
    f8 = mybir.dt.float8e4
    i16 = mybir.dt.int16
    RELU = mybir.ActivationFunctionType.Relu
    COPY = mybir.ActivationFunctionType.Copy

    nc = bacc.Bacc("TRN2", target_bir_lowering=False, debug=False,
                   num_devices=NCORES)
    t_prot = nc.dram_tensor("prot", [P, D_ESM], bf, kind="ExternalInput")
    t_ipw = nc.dram_tensor("ipw", [IP, D], f8, kind="ExternalInput")
    t_Wesm = nc.dram_tensor("W_esmT", [128, KE, D], bf, kind="ExternalInput")
    t_Wupd = nc.dram_tensor("W_updT", [G, 128, KU, D], bf, kind="ExternalInput")
    t_Wout = nc.dram_tensor("W_outT", [128, 18, LP], bf, kind="ExternalInput")
    t_cbias = nc.dram_tensor("cbias", [1, 128 + 3 * D], bf, kind="ExternalInput")
    t_cbT = nc.dram_tensor("cbT", [128, 8 + NLS], f32, kind="ExternalInput")
    t_esmi = nc.dram_tensor("esm_idx", [128, NS // 16], i16, kind="ExternalInput")
    t_tgti = nc.dram_tensor("tgt_idx", [128, NS // 16], i16, kind="ExternalInput")
    t_eidx = nc.dram_tensor("e_idx", [128, TOTC_E * 8], i16, kind="ExternalInput")
    t_bidx = nc.dram_tensor("b_idx", [128, TOTC_B * 8], i16, kind="ExternalInput")
    t_selfS = nc.dram_tensor("sel_self", [128, TOTC_E * 128], f8, kind="ExternalInput")
    t_ppiS = nc.dram_tensor("sel_ppi", [128, TOTC_E * 128], f8, kind="ExternalInput")
    t_bagS = nc.dram_tensor("sel_bag", [128, TOTC_B * 128], f8, kind="ExternalInput")

    if PHASES >= 3:
        t_out = nc.dram_tensor("out", [LP, NS], f32, kind="ExternalOutput")
    elif PHASES == 2:
        t_out = nc.dram_tensor("out", [D, NS], f32, kind="ExternalOutput")
    else:
        t_out = nc.dram_tensor("out", [NS, D], f32, kind="ExternalOutput")

    def blk_ranges(CH):
        r, c0 = [], 0
        for b in range(NBLK):
            r.append((c0, c0 + CH[b]))
            c0 += CH[b]
        return r

    BR_E = blk_ranges(CH_E)
    BR_B = blk_ranges(CH_B)
    U_E = _units(TOTC_E)
    U_B = _units(TOTC_B)

    with tile.TileContext(nc) as tc:
        with (
            tc.tile_pool(name="static", bufs=1) as stat,
            tc.tile_pool(name="dram", bufs=1, space="DRAM") as dram,
        ):
            any_bias = (meta["has_bias_x1"] or meta["has_bias_x2"]
                        or meta["has_bias_upd"] or meta["has_bias_out"])
            if any_bias:
                cb = stat.tile([1, 128 + 3 * D], bf)
                nc.sync.dma_start(cb[:], t_cbias[:])
                cbT_s = stat.tile([128, 8 + NLS], f32)
                nc.sync.dma_start(cbT_s[:], t_cbT[:])
                ones = cb[0:1, 0:128]
            else:
                cb = None
                cbT_s = None
                ones = None
            eidx_s = stat.tile([128, TOTC_E * 8], i16)
            nc.sync.dma_start(eidx_s[:], t_eidx[:])
            # target-protein rows (transposed), used by the esm-final parts
            esmtT = stat.tile([128, KE, NS], bf)

            # DRAM intermediates
            hb = [None, None]
            hf = [None, None]
            for li in range(2):
                hf[li] = dram.tile([N, D], f8, tag=f"hf{li}", name=f"hf{li}",
                                   addr_space="Shared")
                hb[li] = dram.tile([NS, D], f8, tag=f"hb{li}", name=f"hb{li}")
            partial = dram.tile([LP, NS], bf, name="partial")

            def emit_allgather(li):
                nc.gpsimd.collective_compute(
                    "AllGather", mybir.AluOpType.bypass,
                    replica_groups=[list(range(NCORES))],
                    ins=[hb[li][:].opt()],
                    outs=[hf[li][:].opt()],
                )

            def emit_lsub(wp, fp, pse, wt_cache, ls):
                """Emit one esm-partial L-sub (two node-halves)."""
                lg = (ls // 4) * 4
                if lg not in wt_cache:
                    wt = wp.tile([128, KE, 512], bf, tag="we")
                    nc.sync.dma_start(
                        wt[:], t_Wout[:, 8:18, lg * 128 : (lg + 4) * 128]
                    )
                    wt_cache.clear()
                    wt_cache[lg] = wt
                wt = wt_cache[lg]
                li = ls - lg
                for half in range(2):
                    ps = pse.tile([128, D], f32, tag="pse")
                    for kk in range(KE):
                        lhsT = wt[:, kk, li * 128 : (li + 1) * 128]
                        for ng in range(2):
                            nc.tensor.matmul(
                                ps[:, ng * 512 : (ng + 1) * 512],
                                lhsT,
                                esmtT[:, kk,
                                      half * D + ng * 512
                                      : half * D + (ng + 1) * 512],
                                start=(kk == 0), stop=(kk == KE - 1),
                            )
                    pt = fp.tile([128, D], bf, tag="pt")
                    nc.vector.tensor_copy(pt[:], ps[:])
                    nc.sync.dma_start(
                        partial[ls * 128 : (ls + 1) * 128,
                                half * D : (half + 1) * D],
                        pt[:],
                    )

            def esm_final_part(ls0, ls1):
                """out^T partial for L-subs [ls0, ls1): esm k-chunks only."""
                with (
                    tc.tile_pool(name=f"we{ls0}", bufs=2) as wp,
                    tc.tile_pool(name=f"pf{ls0}", bufs=3) as fp,
                    tc.tile_pool(name=f"psE{ls0}", bufs=2, space="PSUM") as pse,
                ):
                    wt_cache = {}
                    for ls in range(ls0, ls1):
                        emit_lsub(wp, fp, pse, wt_cache, ls)

            # ---------------- Phase A: x1 + x2 -> h0 (fp8, x S_H[0]) -------
            N_INL = 13  # esm-final L-subs interleaved into phase A
            with (
                tc.tile_pool(name="esmT", bufs=1) as esmT_p,
                tc.tile_pool(name="tgg", bufs=2) as tgg_p,
                tc.tile_pool(name="msg", bufs=3) as msg_p,
                tc.tile_pool(name="sel", bufs=3) as sel_p,
                tc.tile_pool(name="hmix", bufs=3) as hmix_p,
                tc.tile_pool(name="weA", bufs=2) as wpA,
                tc.tile_pool(name="pfA", bufs=3) as fpA,
                tc.tile_pool(name="psA", bufs=2, space="PSUM") as psA,
                tc.tile_pool(name="psEA", bufs=2, space="PSUM") as pseA,
            ):
                wtcA = {}
                lsub_done = 0
                Wesm_s = esmT_p.tile([128, KE, D], bf)
                nc.sync.dma_start(Wesm_s[:], t_Wesm[:])
                esmi_s = esmT_p.tile([128, NS // 16], i16)
                nc.sync.dma_start(esmi_s[:], t_esmi[:])
                tgti_s = esmT_p.tile([128, NS // 16], i16)
                nc.sync.dma_start(tgti_s[:], t_tgti[:])
                bidx_s = esmT_p.tile([128, TOTC_B * 8], i16)
                nc.sync.dma_start(bidx_s[:], t_bidx[:])
                esmT = []
                for u in range(NBLK):
                    tl = esmT_p.tile([128, KE, 128], bf, tag="esmT", bufs=NBLK)
                    nc.gpsimd.dma_gather(
                        tl[:], t_prot[:], esmi_s[:, u * 8 : (u + 1) * 8],
                        128, 128, D_ESM, transpose=True, single_packet=False,
                    )
                    esmT.append(tl)

                # target-protein gathers for the esm-final trick (early so
                # interleaved esm-final L-subs can start mid-phase)
                for u in range(NBLK):
                    tg = tgg_p.tile([128, KE, 128], bf, tag="tg")
                    nc.gpsimd.dma_gather(
                        tg[:], t_prot[:], tgti_s[:, u * 8 : (u + 1) * 8],
                        128, 128, D_ESM, transpose=True,
                    )
                    nc.vector.tensor_copy(
                        esmtT[:, :, u * 128 : (u + 1) * 128], tg[:]
                    )

                # bag gathers (issued early; consumed per-chunk below)
                bmsg = {}
                bsel = {}
                for ui, (c0, nch) in enumerate(U_B):
                    mt = msg_p.tile([128, 8, D], f8, tag="msg")
                    nc.gpsimd.dma_gather(
                        mt[:, 0:nch, :], t_ipw[:],
                        bidx_s[:, c0 * 8 : (c0 + nch) * 8],
                        nch * 128, nch * 128, D, single_packet=False,
                    )
                    st = sel_p.tile([128, 8, 128], f8, tag="sel")
                    nc.sync.dma_start(
                        st[:, 0:nch, :],
                        t_bagS[:, c0 * 128 : (c0 + nch) * 128],
                    )
                    bmsg[ui] = mt
                    bsel[ui] = st

                for nt in range(NBLK):
                    # x1 psum
                    ps1 = psA.tile([128, D], f32, tag="ps")
                    for jj in range(KE):
                        for b in range(2):
                            nc.tensor.matmul(
                                ps1[:, b * 512 : (b + 1) * 512],
                                esmT[nt][:, jj, :],
                                Wesm_s[:, jj, b * 512 : (b + 1) * 512],
                                start=(jj == 0),
                                stop=(jj == KE - 1 and not meta["has_bias_x1"]),
                            )
                    if meta["has_bias_x1"]:
                        for b in range(2):
                            nc.tensor.matmul(
                                ps1[:, b * 512 : (b + 1) * 512], ones,
                                cb[0:1, 128 + b * 512 : 128 + (b + 1) * 512],
                                start=False, stop=True,
                            )
                    # x2 psum: selector matmuls over this block's chunks
                    ps2 = psA.tile([128, D], f32, tag="ps")
                    c0, c1 = BR_B[nt]
                    prs = _pairs(c0, c1)
                    for pi, (ci, n) in enumerate(prs):
                        mt, st = bmsg[ci // 8], bsel[ci // 8]
                        j = ci % 8
                        first = pi == 0
                        last = (pi == len(prs) - 1
                                and not meta["has_bias_x2"])
                        for b in range(2):
                            if n == 2:
                                nc.tensor.matmul(
                                    ps2[:, b * 512 : (b + 1) * 512],
                                    st[:, j : j + 2, :],
                                    mt[:, j : j + 2, b * 512 : (b + 1) * 512],
                                    start=first, stop=last,
                                    perf_mode=mybir.MatmulPerfMode.DoubleRow,
                                )
                            else:
                                nc.tensor.matmul(
                                    ps2[:, b * 512 : (b + 1) * 512],
                                    st[:, j, :],
                                    mt[:, j, b * 512 : (b + 1) * 512],
                                    start=first, stop=last,
                                )
                    if meta["has_bias_x2"]:
                        for b in range(2):
                            nc.tensor.matmul(
                                ps2[:, b * 512 : (b + 1) * 512], ones,
                                cb[0:1, 128 + D + b * 512 : 128 + D + (b + 1) * 512],
                                start=False, stop=True,
                            )
                    # mix (scaled by S_H[0]): h8 = sm0*S*relu(x1) + sm1*S*relu(x2)
                    m1 = hmix_p.tile([128, D], bf, tag="m1")
                    m2 = hmix_p.tile([128, D], bf, tag="m2")
                    h8 = hmix_p.tile([128, D], f8, tag="h8")
                    nc.scalar.activation(m1[:], ps1[:], RELU, scale=sm0 * S_H[0])
                    nc.scalar.activation(
                        m2[:], ps2[:], RELU, scale=sm1 * S_H[0] / S_IPW
                    )
                    nc.vector.tensor_add(h8[:], m1[:], m2[:])
                    if PHASES <= 1:
                        hf32 = hmix_p.tile([128, D], f32, tag="hf32")
                        nc.scalar.activation(
                            hf32[:], h8[:], COPY, scale=1.0 / S_H[0]
                        )
                        nc.sync.dma_start(
                            t_out[nt * 128 : (nt + 1) * 128, :], hf32[:]
                        )
                    nc.sync.dma_start(
                        hb[0][nt * 128 : (nt + 1) * 128, :], h8[:]
                    )
                    if nt >= 2:
                        want = (nt - 1) * N_INL // (NBLK - 2)
                        while lsub_done < min(want, N_INL):
                            emit_lsub(wpA, fpA, pseA, wtcA, lsub_done)
                            lsub_done += 1
                    if nt == NBLK - 1:
                        emit_allgather(0)

            if PHASES <= 1:
                return nc

            # esm-final part 1 fills the h0 AllGather wall
            esm_final_part(13, 23)

            # ---------------- GNN layers ----------------
            def segsum_layer(layer, catT_all):
                mbufs = 5
                with (
                    tc.tile_pool(name=f"msgL{layer}", bufs=mbufs) as msg_p,
                    tc.tile_pool(name=f"selL{layer}", bufs=3) as sel_p,
                    tc.tile_pool(name=f"catL{layer}", bufs=2) as cat_p,
                    tc.tile_pool(name=f"psS{layer}", bufs=2, space="PSUM") as ps_p,
                ):
                    emsg, esel_s, esel_p = {}, {}, {}
                    for ui, (c0, nch) in enumerate(U_E):
                        mt = msg_p.tile([128, 8, D], f8, tag="msg")
                        nc.gpsimd.dma_gather(
                            mt[:, 0:nch, :], hf[layer][:],
                            eidx_s[:, c0 * 8 : (c0 + nch) * 8],
                            nch * 128, nch * 128, D, single_packet=False,
                        )
                        s1 = sel_p.tile([128, 8, 128], f8, tag="sself")
                        nc.sync.dma_start(
                            s1[:, 0:nch, :],
                            t_selfS[:, c0 * 128 : (c0 + nch) * 128],
                        )
                        s2 = sel_p.tile([128, 8, 128], f8, tag="sppi")
                        nc.sync.dma_start(
                            s2[:, 0:nch, :],
                            t_ppiS[:, c0 * 128 : (c0 + nch) * 128],
                        )
                        emsg[ui], esel_s[ui], esel_p[ui] = mt, s1, s2

                    inv = 1.0 / S_H[layer]
                    for blk in range(NBLK):
                        # cat psum: cols [0:D]=ppi, [D:2D]=res
                        cps = ps_p.tile([128, 2 * D], f32, tag="cps")
                        c0, c1 = BR_E[blk]
                        prs = _pairs(c0, c1)
                        for pi, (ci, n) in enumerate(prs):
                            mt = emsg[ci // 8]
                            j = ci % 8
                            first = pi == 0
                            last = pi == len(prs) - 1
                            sp = esel_p[ci // 8]
                            ss = esel_s[ci // 8]
                            for off, sel in ((0, sp), (D, ss)):
                                for b in range(2):
                                    if n == 2:
                                        nc.tensor.matmul(
                                            cps[:, off + b * 512
                                                : off + (b + 1) * 512],
                                            sel[:, j : j + 2, :],
                                            mt[:, j : j + 2,
                                               b * 512 : (b + 1) * 512],
                                            start=first, stop=last,
                                            perf_mode=(
                                                mybir.MatmulPerfMode.DoubleRow
                                            ),
                                        )
                                    else:
                                        nc.tensor.matmul(
                                            cps[:, off + b * 512
                                                : off + (b + 1) * 512],
                                            sel[:, j, :],
                                            mt[:, j, b * 512 : (b + 1) * 512],
                                            start=first, stop=last,
                                        )
                        catt = cat_p.tile([128, 2 * D], bf, tag="cat")
                        nc.scalar.activation(
                            catt[:, 0:D], cps[:, 0:D], COPY, scale=inv
                        )
                        nc.scalar.activation(
                            catt[:, D : 2 * D], cps[:, D : 2 * D], COPY,
                            scale=inv,
                        )
                        nc.sync.dma_start_transpose(
                            catT_all[:, :, blk * 128 : (blk + 1) * 128], catt[:]
                        )

            # ---- layer 0 ----
            with tc.tile_pool(name="l0", bufs=1) as l0_p:
                catT0 = l0_p.tile([128, KU, NS], bf)
                segsum_layer(0, catT0)
                with (
                    tc.tile_pool(name="upd0", bufs=1) as upd_p,
                    tc.tile_pool(name="hn0", bufs=3) as hn_p,
                    tc.tile_pool(name="psU0", bufs=2, space="PSUM") as psU,
                ):
                    Wu = upd_p.tile([128, KU, D], bf)
                    nc.sync.dma_start(Wu[:], t_Wupd[0])
                    for blk in range(NBLK):
                        psu = psU.tile([128, D], f32, tag="psu")
                        for kk in range(KU):
                            for b in range(2):
                                nc.tensor.matmul(
                                    psu[:, b * 512 : (b + 1) * 512],
                                    catT0[:, kk, blk * 128 : (blk + 1) * 128],
                                    Wu[:, kk, b * 512 : (b + 1) * 512],
                                    start=(kk == 0),
                                    stop=(kk == KU - 1
                                          and not meta["has_bias_upd"]),
                                )
                        if meta["has_bias_upd"]:
                            for b in range(2):
                                nc.tensor.matmul(
                                    psu[:, b * 512 : (b + 1) * 512], ones,
                                    cb[0:1, 128 + 2 * D + b * 512
                                       : 128 + 2 * D + (b + 1) * 512],
                                    start=False, stop=True,
                                )
                        ht = hn_p.tile([128, D], f8, tag="h")
                        nc.scalar.activation(ht[:], psu[:], RELU, scale=S_H[1])
                        nc.sync.dma_start(
                            hb[1][blk * 128 : (blk + 1) * 128, :], ht[:]
                        )
                        if blk == NBLK - 1:
                            emit_allgather(1)

            # esm-final part 2 fills the h1 AllGather wall
            esm_final_part(23, NLS)

            # ---- layer 1 (h2^T stays in SBUF) + final ----
            with tc.tile_pool(name="late", bufs=1) as late_p:
                h2T = late_p.tile([128, D // 128, NS], bf)
                with tc.tile_pool(name="l1", bufs=1) as l1_p:
                    catT1 = l1_p.tile([128, KU, NS], bf)
                    segsum_layer(1, catT1)
                    with (
                        tc.tile_pool(name="upd1", bufs=1) as upd_p,
                        tc.tile_pool(name="psU1", bufs=2, space="PSUM") as psU,
                    ):
                        Wu = upd_p.tile([128, KU, D], bf)
                        nc.sync.dma_start(Wu[:], t_Wupd[1])
                        for ds in range(D // 128):
                            psu = psU.tile([128, NS], f32, tag="psu2")
                            for kk in range(KU):
                                lhsT = Wu[:, kk, ds * 128 : (ds + 1) * 128]
                                for ng in range(4):
                                    nc.tensor.matmul(
                                        psu[:, ng * 512 : (ng + 1) * 512],
                                        lhsT,
                                        catT1[:, kk, ng * 512 : (ng + 1) * 512],
                                        start=(kk == 0), stop=(kk == KU - 1),
                                    )
                            if meta["has_bias_upd"]:
                                nc.scalar.activation(
                                    h2T[:, ds, :], psu[:], RELU,
                                    bias=cbT_s[:, ds : ds + 1],
                                )
                            else:
                                nc.scalar.activation(h2T[:, ds, :], psu[:], RELU)

                if PHASES == 2:
                    with tc.tile_pool(name="dbg", bufs=4) as dbg_p:
                        for ds in range(D // 128):
                            ff = dbg_p.tile([128, NS], f32, tag="ff")
                            nc.vector.tensor_copy(ff[:], h2T[:, ds, :])
                            nc.sync.dma_start(
                                t_out[ds * 128 : (ds + 1) * 128, :], ff[:])
                    return nc

                # ---- Final: out^T = W_h2-chunks^T @ h2T + partial ----
                with (
                    tc.tile_pool(name="wout", bufs=2) as wout_p,
                    tc.tile_pool(name="fin", bufs=3) as fin_p,
                    tc.tile_pool(name="psF", bufs=2, space="PSUM") as psF,
                ):
                    for lg in range(0, NLS, 4):
                        wt = wout_p.tile([128, 8, 512], bf, tag="wo")
                        nc.sync.dma_start(
                            wt[:], t_Wout[:, 0:8, lg * 128 : (lg + 4) * 128]
                        )
                        for li in range(4):
                            ls = lg + li
                            ps = psF.tile([128, NS], f32, tag="psf")
                            for kk in range(8):
                                lhsT = wt[:, kk, li * 128 : (li + 1) * 128]
                                for ng in range(4):
                                    nc.tensor.matmul(
                                        ps[:, ng * 512 : (ng + 1) * 512],
                                        lhsT,
                                        h2T[:, kk, ng * 512 : (ng + 1) * 512],
                                        start=(kk == 0), stop=(kk == 7),
                                    )
                            pt = fin_p.tile([128, NS], bf, tag="pt")
                            nc.sync.dma_start(
                                pt[:], partial[ls * 128 : (ls + 1) * 128, :]
                            )
                            ot = fin_p.tile([128, NS], f32, tag="o")
                            nc.vector.tensor_add(ot[:], ps[:], pt[:])
                            if meta["has_bias_out"]:
                                nc.vector.tensor_scalar_add(
                                    ot[:], ot[:], cbT_s[:, 8 + ls : 8 + ls + 1]
                                )
                            nc.sync.dma_start(
                                t_out[ls * 128 : (ls + 1) * 128, :], ot[:]
                            )
    return nc


def kernel(**inputs):
    meta, in_maps = preprocess(inputs)
    nc = build(meta)
    nc.compile()
    res = bass_utils.run_bass_kernel_spmd(
        nc, in_maps, core_ids=list(range(NCORES)), trace=TRACE
    )
    kernel.last_exec_ns = res.exec_time_ns
    if PHASES >= 3:
        out = np.concatenate(
            [res.results[c]["out"][:L, :].T for c in range(NCORES)], axis=0
        )
        out = np.ascontiguousarray(out)
    elif PHASES == 2:
        kernel.per_core = [
            np.ascontiguousarray(res.results[c]["out"].T) for c in range(NCORES)
        ]
        out = kernel.per_core[0]
    else:
        out = res.results[0]["out"]
        kernel.per_core = [res.results[c]["out"] for c in range(NCORES)]
    return out


# revision 19
# speedup vs baseline: 1.0209x; 1.0209x over previous
"""Trainium2 Bass kernel for the GNN message-passing network.

Sharding: 16384 nodes split across 8 NeuronCores (2048 nodes/core).
Tables and weights are replicated; per-core index/selector tensors drive
dma_gather row gathers and selector-matmul segment sums (PSUM fp32).

Key optimizations:
- h0/h1 are exchanged and gathered in fp8e4 (scaled by 256/64), and the
  segment-sum selector matmuls run fp8 x fp8; the interpro table is
  gathered in fp8 (x16). Measured end-to-end rel err ~1.5e-2 (< 2e-2).
- The esm half of the final matmul (target-protein rows x W_out[:,D:])
  only depends on a gather, so it runs inside the two AllGather walls
  where the tensor engine would otherwise idle; partial outputs park in
  DRAM (bf16) and are added back in the final phase.
- cat matrices are transposed on-chip (SBUF->SBUF DMA transpose); the
  layer-1 update and final matmul run output-transposed with stationary
  weight tiles, so h2^T stays SBUF-resident. Output is [L, nodes] per
  core; the host transposes.
"""
import numpy as np
import ml_dtypes

import concourse.bacc as bacc
import concourse.mybir as mybir
import concourse.tile as tile
from concourse import bass_utils

BF16 = ml_dtypes.bfloat16
FP8 = ml_dtypes.float8_e4m3

# Problem shapes (fixed).
N = 16384
E = 262144
T = 327680
P = 20000
IP = 30000
D_ESM = 1280
D = 1024
L = 5000
LP = 5120                 # L padded to 128 multiple
NLS = LP // 128           # 40 L-sub tiles
G = 2
NCORES = 8
NS = N // NCORES          # 2048 nodes per core
NBLK = NS // 128          # 16 dst blocks per core
KE = D_ESM // 128         # 10 k-chunks for esm
KU = (2 * D) // 128       # 16 k-chunks for update matmul
S_IPW = 16.0              # interpro table fp8 scale
S_H = [256.0, 64.0]       # h0 / h1 fp8 scales

# Set to 0/2 to truncate the kernel for debugging (test.py uses this).
PHASES = 3
TRACE = False


def _wrap_idx(idx, total):
    """[128, total/16] int16: token i at (i%16, i//16), replicated x8 groups."""
    a = np.zeros(total, np.int16)
    a[: len(idx)] = idx.astype(np.int16)
    blk = a.reshape(total // 16, 16).T
    return np.tile(blk, (8, 1)).copy()


def _pack_stream(tok_idx_per_block, dcol_per_block, val_per_block, ch_per_block):
    """Build padded token stream + selector array for one core."""
    tot = sum(ch_per_block) * 128
    idx_s = np.zeros(tot, np.int64)
    pos_l = []
    col_l = []
    val_l = []
    base = 0
    for b in range(len(ch_per_block)):
        tok = tok_idx_per_block[b]
        n = len(tok)
        idx_s[base : base + n] = tok
        pos = base + np.arange(n)
        pos_l.append(pos)
        col_l.append(dcol_per_block[b])
        val_l.append(
            val_per_block[b] if val_per_block is not None else np.ones(n, np.float32)
        )
        base += ch_per_block[b] * 128
    pos = np.concatenate(pos_l) if pos_l else np.zeros(0, np.int64)
    col = np.concatenate(col_l).astype(np.int64) if col_l else np.zeros(0, np.int64)
    val = np.concatenate(val_l) if val_l else np.zeros(0, np.float32)
    return idx_s, pos, col, val


def _sel_array(pos, col, val, totc):
    """[128, totc*128] fp8 selector: S[pos%128, (pos//128)*128 + col] = val."""
    sel = np.zeros((128, totc * 128), np.float32)
    sel[pos % 128, (pos // 128) * 128 + col] = val
    return sel.astype(FP8)


def _pairs(c0, c1):
    """DoubleRow pairing measured slower on HW; emit singles."""
    return [(ci, 1) for ci in range(c0, c1)]


def _units(totc):
    """Split totc 128-token chunks into gather units of <=8 chunks."""
    out = []
    c0 = 0
    while c0 < totc:
        n = min(8, totc - c0)
        out.append((c0, n))
        c0 += n
    return out


def preprocess(inputs):
    """Host-side: shard, sort edges by dst, build index/selector tensors."""
    prot = np.asarray(inputs["protein_embedding"], np.float32)
    ipw = np.asarray(inputs["interpro_weight"], np.float32)
    W_esm = np.asarray(inputs["W_esm"], np.float32)
    b_esm = np.asarray(inputs["b_esm"], np.float32)
    bias1 = np.asarray(inputs["bias1"], np.float32)
    bias2 = np.asarray(inputs["bias2"], np.float32)
    w = np.asarray(inputs["w"], np.float32)
    W_upd = np.asarray(inputs["W_upd"], np.float32)
    b_upd = np.asarray(inputs["b_upd"], np.float32)
    W_out = np.asarray(inputs["W_out"], np.float32)
    b_out = np.asarray(inputs["b_out"], np.float32)
    self_w = np.asarray(inputs["self_w"], np.float32)
    ppi_w = np.asarray(inputs["ppi_w"], np.float32)
    node_in = np.asarray(inputs["inputs"], np.int64)
    ip_idx = np.asarray(inputs["interpro_idx"], np.int64)
    ip_off = np.asarray(inputs["interpro_off"], np.int64)
    src = np.asarray(inputs["src"], np.int64)
    dst = np.asarray(inputs["dst"], np.int64)
    target = np.asarray(inputs["target_id"], np.int64)

    ew = np.exp(w - w.max())
    sm = ew / ew.sum()

    bias_x1 = b_esm + bias1

    # --- edges: per (core, block) token lists sorted by dst ---
    order = np.argsort(dst, kind="stable")
    src_s, dst_s = src[order], dst[order]
    sw_s, pw_s = self_w[order], ppi_w[order]
    gblk = dst_s // 128
    blk_counts = np.bincount(gblk, minlength=N // 128)
    blk_starts = np.concatenate([[0], np.cumsum(blk_counts)])
    ch_e = np.zeros((NCORES, NBLK), np.int64)
    for c in range(NCORES):
        for b in range(NBLK):
            ch_e[c, b] = -(-blk_counts[c * NBLK + b] // 128)
    CH_E = ch_e.max(axis=0)
    TOTC_E = int(CH_E.sum())

    # --- bags ---
    bag_sizes = ip_off[1:] - ip_off[:-1]
    ch_b = np.zeros((NCORES, NBLK), np.int64)
    for c in range(NCORES):
        for b in range(NBLK):
            n0 = c * NS + b * 128
            cnt = int(ip_off[n0 + 128] - ip_off[n0])
            ch_b[c, b] = max(1, -(-cnt // 128))
    CH_B = ch_b.max(axis=0)
    TOTC_B = int(CH_B.sum())

    meta = dict(
        sm0=float(sm[0]),
        sm1=float(sm[1]),
        CH_E=[int(x) for x in CH_E],
        CH_B=[int(x) for x in CH_B],
        has_bias_x1=bool(np.any(bias_x1 != 0)),
        has_bias_x2=bool(np.any(bias2 != 0)),
        has_bias_upd=bool(np.any(b_upd != 0)),
        has_bias_out=bool(np.any(b_out != 0)),
    )

    W_esmT = np.ascontiguousarray(
        W_esm.T.reshape(KE, 128, D).transpose(1, 0, 2)
    ).astype(BF16)  # [128, KE, D]
    W_updT = np.ascontiguousarray(
        W_upd.transpose(0, 2, 1).reshape(G, KU, 128, D).transpose(0, 2, 1, 3)
    ).astype(BF16)  # [G, 128, KU, D]
    W_outP = np.zeros((18 * 128, LP), np.float32)
    W_outP[: D + D_ESM, :L] = W_out.T
    W_outT = np.ascontiguousarray(
        W_outP.reshape(18, 128, LP).transpose(1, 0, 2)
    ).astype(BF16)  # [128, 18, LP]
    # row-oriented biases (free-dim broadcast via ones-matmul)
    cbias = np.zeros((1, 128 + 3 * D), np.float32)
    cbias[0, :128] = 1.0
    cbias[0, 128 : 128 + D] = bias_x1
    cbias[0, 128 + D : 128 + 2 * D] = bias2
    cbias[0, 128 + 2 * D : 128 + 3 * D] = b_upd[0]
    # partition-oriented biases for transposed outputs (scalar bias add)
    cbT = np.zeros((128, 8 + NLS), np.float32)
    cbT[:, 0:8] = b_upd[1].reshape(8, 128).T
    cbT[:, 8 : 8 + NLS] = np.pad(b_out, (0, LP - L)).reshape(NLS, 128).T
    shared = dict(
        prot=prot.astype(BF16),
        ipw=(ipw * S_IPW).astype(FP8),
        W_esmT=W_esmT,
        W_updT=W_updT,
        W_outT=W_outT,
        cbias=cbias.astype(BF16),
        cbT=cbT.astype(np.float32),
    )

    in_maps = []
    for c in range(NCORES):
        esm_idx = _wrap_idx(node_in[c * NS : (c + 1) * NS], NS)
        tgt_idx = _wrap_idx(target[c * NS : (c + 1) * NS], NS)

        tokb, colb, valsb, valpb = [], [], [], []
        for b in range(NBLK):
            s0, s1 = blk_starts[c * NBLK + b], blk_starts[c * NBLK + b + 1]
            tokb.append(src_s[s0:s1])
            colb.append(dst_s[s0:s1] - (c * NS + b * 128))
            valsb.append(sw_s[s0:s1])
            valpb.append(pw_s[s0:s1])
        eidx, epos, ecol, esv = _pack_stream(tokb, colb, valsb, list(CH_E))
        _, _, _, epv = _pack_stream(tokb, colb, valpb, list(CH_E))
        sel_self = _sel_array(epos, ecol, esv, TOTC_E)
        sel_ppi = _sel_array(epos, ecol, epv, TOTC_E)

        tokb, colb = [], []
        for b in range(NBLK):
            n0 = c * NS + b * 128
            i0, i1 = int(ip_off[n0]), int(ip_off[n0 + 128])
            tokb.append(ip_idx[i0:i1])
            colb.append(
                np.repeat(
                    np.arange(128), bag_sizes[n0 : n0 + 128].astype(np.int64)
                )
            )
        bidx, bpos, bcol, bval = _pack_stream(tokb, colb, None, list(CH_B))
        sel_bag = _sel_array(bpos, bcol, bval, TOTC_B)

        m = dict(shared)
        m.update(
            esm_idx=esm_idx,
            tgt_idx=tgt_idx,
            e_idx=_wrap_idx(eidx, TOTC_E * 128),
            b_idx=_wrap_idx(bidx, TOTC_B * 128),
            sel_self=sel_self,
            sel_ppi=sel_ppi,
            sel_bag=sel_bag,
        )
        in_maps.append(m)
    return meta, in_maps


def build(meta):
    CH_E = meta["CH_E"]
    CH_B = meta["CH_B"]
    TOTC_E = sum(CH_E)
    TOTC_B = sum(CH_B)
    sm0, sm1 = meta["sm0"], meta["sm1"]
    bf = mybir.dt.bfloat16
    f32 = mybir.dt.float32
    f8 = mybir.dt.float8e4
    i16 = mybir.dt.int16
    RELU = mybir.ActivationFunctionType.Relu
    COPY = mybir.ActivationFunctionType.Copy

    nc = bacc.Bacc("TRN2", target_bir_lowering=False, debug=False,
                   num_devices=NCORES)
    t_prot = nc.dram_tensor("prot", [P, D_ESM], bf, kind="ExternalInput")
    t_ipw = nc.dram_tensor("ipw", [IP, D], f8, kind="ExternalInput")
    t_Wesm = nc.dram_tensor("W_esmT", [128, KE, D], bf, kind="ExternalInput")
    t_Wupd = nc.dram_tensor("W_updT", [G, 128, KU, D], bf, kind="ExternalInput")
    t_Wout = nc.dram_tensor("W_outT", [128, 18, LP], bf, kind="ExternalInput")
    t_cbias = nc.dram_tensor("cbias", [1, 128 + 3 * D], bf, kind="ExternalInput")
    t_cbT = nc.dram_tensor("cbT", [128, 8 + NLS], f32, kind="ExternalInput")
    t_esmi = nc.dram_tensor("esm_idx", [128, NS // 16], i16, kind="ExternalInput")
    t_tgti = nc.dram_tensor("tgt_idx", [128, NS // 16], i16, kind="ExternalInput")
    t_eidx = nc.dram_tensor("e_idx", [128, TOTC_E * 8], i16, kind="ExternalInput")
    t_bidx = nc.dram_tensor("b_idx", [128, TOTC_B * 8], i16, kind="ExternalInput")
    t_selfS = nc.dram_tensor("sel_self", [128, TOTC_E * 128], f8, kind="ExternalInput")
    t_ppiS = nc.dram_tensor("sel_ppi", [128, TOTC_E * 128], f8, kind="ExternalInput")
    t_bagS = nc.dram_tensor("sel_bag", [128, TOTC_B * 128], f8, kind="ExternalInput")

    if PHASES >= 3:
        t_out = nc.dram_tensor("out", [LP, NS], f32, kind="ExternalOutput")
    elif PHASES == 2:
        t_out = nc.dram_tensor("out", [D, NS], f32, kind="ExternalOutput")
    else:
        t_out = nc.dram_tensor("out", [NS, D], f32, kind="ExternalOutput")

    def blk_ranges(CH):
        r, c0 = [], 0
        for b in range(NBLK):
            r.append((c0, c0 + CH[b]))
            c0 += CH[b]
        return r

    BR_E = blk_ranges(CH_E)
    BR_B = blk_ranges(CH_B)
    U_E = _units(TOTC_E)
    U_B = _units(TOTC_B)

    with tile.TileContext(nc) as tc:
        with (
            tc.tile_pool(name="static", bufs=1) as stat,
            tc.tile_pool(name="dram", bufs=1, space="DRAM") as dram,
        ):
            any_bias = (meta["has_bias_x1"] or meta["has_bias_x2"]
                        or meta["has_bias_upd"] or meta["has_bias_out"])
            if any_bias:
                cb = stat.tile([1, 128 + 3 * D], bf)
                nc.sync.dma_start(cb[:], t_cbias[:])
                cbT_s = stat.tile([128, 8 + NLS], f32)
                nc.sync.dma_start(cbT_s[:], t_cbT[:])
                ones = cb[0:1, 0:128]
            else:
                cb = None
                cbT_s = None
                ones = None
            eidx_s = stat.tile([128, TOTC_E * 8], i16)
            nc.sync.dma_start(eidx_s[:], t_eidx[:])
            # target-protein rows (transposed), used by the esm-final parts
            esmtT = stat.tile([128, KE, NS], bf)

            # DRAM intermediates
            hb = [None, None]
            hf = [None, None]
            for li in range(2):
                hf[li] = dram.tile([N, D], f8, tag=f"hf{li}", name=f"hf{li}",
                                   addr_space="Shared")
                hb[li] = dram.tile([NS, D], f8, tag=f"hb{li}", name=f"hb{li}")
            partial = dram.tile([LP, NS], bf, name="partial")

            def emit_allgather(li):
                nc.gpsimd.collective_compute(
                    "AllGather", mybir.AluOpType.bypass,
                    replica_groups=[list(range(NCORES))],
                    ins=[hb[li][:].opt()],
                    outs=[hf[li][:].opt()],
                )

            def emit_lsub(wp, fp, pse, wt_cache, ls):
                """Emit one esm-partial L-sub (two node-halves)."""
                lg = (ls // 4) * 4
                if lg not in wt_cache:
                    wt = wp.tile([128, KE, 512], bf, tag="we")
                    nc.sync.dma_start(
                        wt[:], t_Wout[:, 8:18, lg * 128 : (lg + 4) * 128]
                    )
                    wt_cache.clear()
                    wt_cache[lg] = wt
                wt = wt_cache[lg]
                li = ls - lg
                for half in range(2):
                    ps = pse.tile([128, D], f32, tag="pse")
                    for kk in range(KE):
                        lhsT = wt[:, kk, li * 128 : (li + 1) * 128]
                        for ng in range(2):
                            nc.tensor.matmul(
                                ps[:, ng * 512 : (ng + 1) * 512],
                                lhsT,
                                esmtT[:, kk,
                                      half * D + ng * 512
                                      : half * D + (ng + 1) * 512],
                                start=(kk == 0), stop=(kk == KE - 1),
                            )
                    pt = fp.tile([128, D], bf, tag="pt")
                    nc.vector.tensor_copy(pt[:], ps[:])
                    nc.sync.dma_start(
                        partial[ls * 128 : (ls + 1) * 128,
                                half * D : (half + 1) * D],
                        pt[:],
                    )

            def esm_final_part(ls0, ls1):
                """out^T partial for L-subs [ls0, ls1): esm k-chunks only."""
                with (
                    tc.tile_pool(name=f"we{ls0}", bufs=2) as wp,
                    tc.tile_pool(name=f"pf{ls0}", bufs=3) as fp,
                    tc.tile_pool(name=f"psE{ls0}", bufs=2, space="PSUM") as pse,
                ):
                    wt_cache = {}
                    for ls in range(ls0, ls1):
                        emit_lsub(wp, fp, pse, wt_cache, ls)

            # ---------------- Phase A: x1 + x2 -> h0 (fp8, x S_H[0]) -------
            N_INL = 13  # esm-final L-subs interleaved into phase A
            with (
                tc.tile_pool(name="esmT", bufs=1) as esmT_p,
                tc.tile_pool(name="tgg", bufs=2) as tgg_p,
                tc.tile_pool(name="msg", bufs=3) as msg_p,
                tc.tile_pool(name="sel", bufs=3) as sel_p,
                tc.tile_pool(name="hmix", bufs=3) as hmix_p,
                tc.tile_pool(name="weA", bufs=2) as wpA,
                tc.tile_pool(name="pfA", bufs=3) as fpA,
                tc.tile_pool(name="psA", bufs=2, space="PSUM") as psA,
                tc.tile_pool(name="psEA", bufs=2, space="PSUM") as pseA,
            ):
                wtcA = {}
                lsub_done = 0
                Wesm_s = esmT_p.tile([128, KE, D], bf)
                nc.sync.dma_start(Wesm_s[:], t_Wesm[:])
                esmi_s = esmT_p.tile([128, NS // 16], i16)
                nc.sync.dma_start(esmi_s[:], t_esmi[:])
                tgti_s = esmT_p.tile([128, NS // 16], i16)
                nc.sync.dma_start(tgti_s[:], t_tgti[:])
                bidx_s = esmT_p.tile([128, TOTC_B * 8], i16)
                nc.sync.dma_start(bidx_s[:], t_bidx[:])
                esmT = []
                for u in range(NBLK):
                    tl = esmT_p.tile([128, KE, 128], bf, tag="esmT", bufs=NBLK)
                    nc.gpsimd.dma_gather(
                        tl[:], t_prot[:], esmi_s[:, u * 8 : (u + 1) * 8],
                        128, 128, D_ESM, transpose=True,
                    )
                    esmT.append(tl)

                # target-protein gathers for the esm-final trick (early so
                # interleaved esm-final L-subs can start mid-phase)
                for u in range(NBLK):
                    tg = tgg_p.tile([128, KE, 128], bf, tag="tg")
                    nc.gpsimd.dma_gather(
                        tg[:], t_prot[:], tgti_s[:, u * 8 : (u + 1) * 8],
                        128, 128, D_ESM, transpose=True,
                    )
                    nc.vector.tensor_copy(
                        esmtT[:, :, u * 128 : (u + 1) * 128], tg[:]
                    )

                # bag gathers (issued early; consumed per-chunk below)
                bmsg = {}
                bsel = {}
                for ui, (c0, nch) in enumerate(U_B):
                    mt = msg_p.tile([128, 8, D], f8, tag="msg")
                    nc.gpsimd.dma_gather(
                        mt[:, 0:nch, :], t_ipw[:],
                        bidx_s[:, c0 * 8 : (c0 + nch) * 8],
                        nch * 128, nch * 128, D, single_packet=False,
                    )
                    st = sel_p.tile([128, 8, 128], f8, tag="sel")
                    nc.sync.dma_start(
                        st[:, 0:nch, :],
                        t_bagS[:, c0 * 128 : (c0 + nch) * 128],
                    )
                    bmsg[ui] = mt
                    bsel[ui] = st

                for nt in range(NBLK):
                    # x1 psum
                    ps1 = psA.tile([128, D], f32, tag="ps")
                    for jj in range(KE):
                        for b in range(2):
                            nc.tensor.matmul(
                                ps1[:, b * 512 : (b + 1) * 512],
                                esmT[nt][:, jj, :],
                                Wesm_s[:, jj, b * 512 : (b + 1) * 512],
                                start=(jj == 0),
                                stop=(jj == KE - 1 and not meta["has_bias_x1"]),
                            )
                    if meta["has_bias_x1"]:
                        for b in range(2):
                            nc.tensor.matmul(
                                ps1[:, b * 512 : (b + 1) * 512], ones,
                                cb[0:1, 128 + b * 512 : 128 + (b + 1) * 512],
                                start=False, stop=True,
                            )
                    # x2 psum: selector matmuls over this block's chunks
                    ps2 = psA.tile([128, D], f32, tag="ps")
                    c0, c1 = BR_B[nt]
                    prs = _pairs(c0, c1)
                    for pi, (ci, n) in enumerate(prs):
                        mt, st = bmsg[ci // 8], bsel[ci // 8]
                        j = ci % 8
                        first = pi == 0
                        last = (pi == len(prs) - 1
                                and not meta["has_bias_x2"])
                        for b in range(2):
                            if n == 2:
                                nc.tensor.matmul(
                                    ps2[:, b * 512 : (b + 1) * 512],
                                    st[:, j : j + 2, :],
                                    mt[:, j : j + 2, b * 512 : (b + 1) * 512],
                                    start=first, stop=last,
                                    perf_mode=mybir.MatmulPerfMode.DoubleRow,
                                )
                            else:
                                nc.tensor.matmul(
                                    ps2[:, b * 512 : (b + 1) * 512],
                                    st[:, j, :],
                                    mt[:, j, b * 512 : (b + 1) * 512],
                                    start=first, stop=last,
                                )
                    if meta["has_bias_x2"]:
                        for b in range(2):
                            nc.tensor.matmul(
                                ps2[:, b * 512 : (b + 1) * 512], ones,
                                cb[0:1, 128 + D + b * 512 : 128 + D + (b + 1) * 512],
                                start=False, stop=True,
                            )
                    # mix (scaled by S_H[0]): h8 = sm0*S*relu(x1) + sm1*S*relu(x2)
                    m1 = hmix_p.tile([128, D], bf, tag="m1")
                    m2 = hmix_p.tile([128, D], bf, tag="m2")
                    h8 = hmix_p.tile([128, D], f8, tag="h8")
                    nc.scalar.activation(m1[:], ps1[:], RELU, scale=sm0 * S_H[0])
                    nc.scalar.activation(
                        m2[:], ps2[:], RELU, scale=sm1 * S_H[0] / S_IPW
                    )
                    nc.vector.tensor_add(h8[:], m1[:], m2[:])
                    if PHASES <= 1:
                        hf32 = hmix_p.tile([128, D], f32, tag="hf32")
                        nc.scalar.activation(
                            hf32[:], h8[:], COPY, scale=1.0 / S_H[0]
                        )
                        nc.sync.dma_start(
                            t_out[nt * 128 : (nt + 1) * 128, :], hf32[:]
                        )
                    nc.sync.dma_start(
                        hb[0][nt * 128 : (nt + 1) * 128, :], h8[:]
                    )
                    if nt >= 2:
                        want = (nt - 1) * N_INL // (NBLK - 2)
                        while lsub_done < min(want, N_INL):
                            emit_lsub(wpA, fpA, pseA, wtcA, lsub_done)
                            lsub_done += 1
                    if nt == NBLK - 1:
                        emit_allgather(0)

            if PHASES <= 1:
                return nc

            # esm-final part 1 fills the h0 AllGather wall
            esm_final_part(13, 23)

            # ---------------- GNN layers ----------------
            def segsum_layer(layer, catT_all):
                mbufs = 5 if layer == 0 else 4
                with (
                    tc.tile_pool(name=f"msgL{layer}", bufs=mbufs) as msg_p,
                    tc.tile_pool(name=f"selL{layer}", bufs=3) as sel_p,
                    tc.tile_pool(name=f"catL{layer}", bufs=2) as cat_p,
                    tc.tile_pool(name=f"psS{layer}", bufs=2, space="PSUM") as ps_p,
                ):
                    emsg, esel_s, esel_p = {}, {}, {}
                    for ui, (c0, nch) in enumerate(U_E):
                        mt = msg_p.tile([128, 8, D], f8, tag="msg")
                        nc.gpsimd.dma_gather(
                            mt[:, 0:nch, :], hf[layer][:],
                            eidx_s[:, c0 * 8 : (c0 + nch) * 8],
                            nch * 128, nch * 128, D, single_packet=False,
                        )
                        s1 = sel_p.tile([128, 8, 128], f8, tag="sself")
                        nc.sync.dma_start(
                            s1[:, 0:nch, :],
                            t_selfS[:, c0 * 128 : (c0 + nch) * 128],
                        )
                        s2 = sel_p.tile([128, 8, 128], f8, tag="sppi")
                        nc.sync.dma_start(
                            s2[:, 0:nch, :],
                            t_ppiS[:, c0 * 128 : (c0 + nch) * 128],
                        )
                        emsg[ui], esel_s[ui], esel_p[ui] = mt, s1, s2

                    inv = 1.0 / S_H[layer]
                    for blk in range(NBLK):
                        # cat psum: cols [0:D]=ppi, [D:2D]=res
                        cps = ps_p.tile([128, 2 * D], f32, tag="cps")
                        c0, c1 = BR_E[blk]
                        prs = _pairs(c0, c1)
                        for pi, (ci, n) in enumerate(prs):
                            mt = emsg[ci // 8]
                            j = ci % 8
                            first = pi == 0
                            last = pi == len(prs) - 1
                            sp = esel_p[ci // 8]
                            ss = esel_s[ci // 8]
                            for off, sel in ((0, sp), (D, ss)):
                                for b in range(2):
                                    if n == 2:
                                        nc.tensor.matmul(
                                            cps[:, off + b * 512
                                                : off + (b + 1) * 512],
                                            sel[:, j : j + 2, :],
                                            mt[:, j : j + 2,
                                               b * 512 : (b + 1) * 512],
                                            start=first, stop=last,
                                            perf_mode=(
                                                mybir.MatmulPerfMode.DoubleRow
                                            ),
                                        )
                                    else:
                                        nc.tensor.matmul(
                                            cps[:, off + b * 512
                                                : off + (b + 1) * 512],
                                            sel[:, j, :],
                                            mt[:, j, b * 512 : (b + 1) * 512],
                                            start=first, stop=last,
                                        )
                        catt = cat_p.tile([128, 2 * D], bf, tag="cat")
                        nc.scalar.activation(
                            catt[:, 0:D], cps[:, 0:D], COPY, scale=inv
                        )
                        nc.scalar.activation(
                            catt[:, D : 2 * D], cps[:, D : 2 * D], COPY,
                            scale=inv,
                        )
                        nc.sync.dma_start_transpose(
                            catT_all[:, :, blk * 128 : (blk + 1) * 128], catt[:]
                        )

            # ---- layer 0 ----
            with tc.tile_pool(name="l0", bufs=1) as l0_p:
                catT0 = l0_p.tile([128, KU, NS], bf)
                segsum_layer(0, catT0)
                with (
                    tc.tile_pool(name="upd0", bufs=1) as upd_p,
                    tc.tile_pool(name="hn0", bufs=3) as hn_p,
                    tc.tile_pool(name="psU0", bufs=2, space="PSUM") as psU,
                ):
                    Wu = upd_p.tile([128, KU, D], bf)
                    nc.sync.dma_start(Wu[:], t_Wupd[0])
                    for blk in range(NBLK):
                        psu = psU.tile([128, D], f32, tag="psu")
                        for kk in range(KU):
                            for b in range(2):
                                nc.tensor.matmul(
                                    psu[:, b * 512 : (b + 1) * 512],
                                    catT0[:, kk, blk * 128 : (blk + 1) * 128],
                                    Wu[:, kk, b * 512 : (b + 1) * 512],
                                    start=(kk == 0),
                                    stop=(kk == KU - 1
                                          and not meta["has_bias_upd"]),
                                )
                        if meta["has_bias_upd"]:
                            for b in range(2):
                                nc.tensor.matmul(
                                    psu[:, b * 512 : (b + 1) * 512], ones,
                                    cb[0:1, 128 + 2 * D + b * 512
                                       : 128 + 2 * D + (b + 1) * 512],
                                    start=False, stop=True,
                                )
                        ht = hn_p.tile([128, D], f8, tag="h")
                        nc.scalar.activation(ht[:], psu[:], RELU, scale=S_H[1])
                        nc.sync.dma_start(
                            hb[1][blk * 128 : (blk + 1) * 128, :], ht[:]
                        )
                        if blk == NBLK - 1:
                            emit_allgather(1)

            # esm-final part 2 fills the h1 AllGather wall
            esm_final_part(23, NLS)

            # ---- layer 1 (h2^T stays in SBUF) + final ----
            with tc.tile_pool(name="late", bufs=1) as late_p:
                h2T = late_p.tile([128, D // 128, NS], bf)
                with tc.tile_pool(name="l1", bufs=1) as l1_p:
                    catT1 = l1_p.tile([128, KU, NS], bf)
                    segsum_layer(1, catT1)
                    with (
                        tc.tile_pool(name="upd1", bufs=1) as upd_p,
                        tc.tile_pool(name="psU1", bufs=2, space="PSUM") as psU,
                    ):
                        Wu = upd_p.tile([128, KU, D], bf)
                        nc.sync.dma_start(Wu[:], t_Wupd[1])
                        for ds in range(D // 128):
                            psu = psU.tile([128, NS], f32, tag="psu2")
                            for kk in range(KU):
                                lhsT = Wu[:, kk, ds * 128 : (ds + 1) * 128]
                                for ng in range(4):
                                    nc.tensor.matmul(
                                        psu[:, ng * 512 : (ng + 1) * 512],
                                        lhsT,
                                        catT1[:, kk, ng * 512 : (ng + 1) * 512],
                                        start=(kk == 0), stop=(kk == KU - 1),
                                    )
                            if meta["has_bias_upd"]:
                                nc.scalar.activation(
                                    h2T[:, ds, :], psu[:], RELU,
                                    bias=cbT_s[:, ds : ds + 1],
                                )
                            else:
                                nc.scalar.activation(h2T[:, ds, :], psu[:], RELU)

                if PHASES == 2:
                    with tc.tile_pool(name="dbg", bufs=4) as dbg_p:
                        for ds in range(D // 128):
                            ff = dbg_p.tile([128, NS], f32, tag="ff")
                            nc.vector.tensor_copy(ff[:], h2T[:, ds, :])
                            nc.sync.dma_start(
                                t_out[ds * 128 : (ds + 1) * 128, :], ff[:])
                    return nc

                # ---- Final: out^T = W_h2-chunks^T @ h2T + partial ----
                with (
                    tc.tile_pool(name="wout", bufs=2) as wout_p,
                    tc.tile_pool(name="fin", bufs=3) as fin_p,
                    tc.tile_pool(name="psF", bufs=2, space="PSUM") as psF,
                ):
                    for lg in range(0, NLS, 4):
                        wt = wout_p.tile([128, 8, 512], bf, tag="wo")
                        nc.sync.dma_start(
                            wt[:], t_Wout[:, 0:8, lg * 128 : (lg + 4) * 128]
                        )
                        for li in range(4):
                            ls = lg + li
                            ps = psF.tile([128, NS], f32, tag="psf")
                            for kk in range(8):
                                lhsT = wt[:, kk, li * 128 : (li + 1) * 128]
                                for ng in range(4):
                                    nc.tensor.matmul(
                                        ps[:, ng * 512 : (ng + 1) * 512],
                                        lhsT,
                                        h2T[:, kk, ng * 512 : (ng + 1) * 512],
                                        start=(kk == 0), stop=(kk == 7),
                                    )
                            pt = fin_p.tile([128, NS], bf, tag="pt")
                            nc.sync.dma_start(
                                pt[:], partial[ls * 128 : (ls + 1) * 128, :]
                            )
                            ot = fin_p.tile([128, NS], f32, tag="o")
                            nc.vector.tensor_add(ot[:], ps[:], pt[:])
                            if meta["has_bias_out"]:
                                nc.vector.tensor_scalar_add(
                                    ot[:], ot[:], cbT_s[:, 8 + ls : 8 + ls + 1]
                                )
                            nc.sync.dma_start(
                                t_out[ls * 128 : (ls + 1) * 128, :], ot[:]
                            )
    return nc


def kernel(**inputs):
    meta, in_maps = preprocess(inputs)
    nc = build(meta)
    nc.compile()
    res = bass_utils.run_bass_kernel_spmd(
        nc, in_maps, core_ids=list(range(NCORES)), trace=TRACE
    )
    kernel.last_exec_ns = res.exec_time_ns
    if PHASES >= 3:
        out = np.concatenate(
            [res.results[c]["out"][:L, :].T for c in range(NCORES)], axis=0
        )
        out = np.ascontiguousarray(out)
    elif PHASES == 2:
        kernel.per_core = [
            np.ascontiguousarray(res.results[c]["out"].T) for c in range(NCORES)
        ]
        out = kernel.per_core[0]
    else:
        out = res.results[0]["out"]
        kernel.per_core = [res.results[c]["out"] for c in range(NCORES)]
    return out


# revision 21
# speedup vs baseline: 1.0423x; 1.0209x over previous
"""Trainium2 Bass kernel for the GNN message-passing network.

Sharding: 16384 nodes split across 8 NeuronCores (2048 nodes/core).
Tables and weights are replicated; per-core index/selector tensors drive
dma_gather row gathers and selector-matmul segment sums (PSUM fp32).

Key optimizations:
- h0/h1 are exchanged and gathered in fp8e4 (scaled by 256/64), and the
  segment-sum selector matmuls run fp8 x fp8; the interpro table is
  gathered in fp8 (x16). Measured end-to-end rel err ~1.5e-2 (< 2e-2).
- The esm half of the final matmul (target-protein rows x W_out[:,D:])
  only depends on a gather, so it runs inside the two AllGather walls
  where the tensor engine would otherwise idle; partial outputs park in
  DRAM (bf16) and are added back in the final phase.
- cat matrices are transposed on-chip (SBUF->SBUF DMA transpose); the
  layer-1 update and final matmul run output-transposed with stationary
  weight tiles, so h2^T stays SBUF-resident. Output is [L, nodes] per
  core; the host transposes.
"""
import numpy as np
import ml_dtypes

import concourse.bacc as bacc
import concourse.mybir as mybir
import concourse.tile as tile
from concourse import bass_utils

BF16 = ml_dtypes.bfloat16
FP8 = ml_dtypes.float8_e4m3

# Problem shapes (fixed).
N = 16384
E = 262144
T = 327680
P = 20000
IP = 30000
D_ESM = 1280
D = 1024
L = 5000
LP = 5120                 # L padded to 128 multiple
NLS = LP // 128           # 40 L-sub tiles
G = 2
NCORES = 8
NS = N // NCORES          # 2048 nodes per core
NBLK = NS // 128          # 16 dst blocks per core
KE = D_ESM // 128         # 10 k-chunks for esm
KU = (2 * D) // 128       # 16 k-chunks for update matmul
S_IPW = 16.0              # interpro table fp8 scale
S_H = [256.0, 64.0]       # h0 / h1 fp8 scales

# Set to 0/2 to truncate the kernel for debugging (test.py uses this).
PHASES = 3
TRACE = False


def _wrap_idx(idx, total):
    """[128, total/16] int16: token i at (i%16, i//16), replicated x8 groups."""
    a = np.zeros(total, np.int16)
    a[: len(idx)] = idx.astype(np.int16)
    blk = a.reshape(total // 16, 16).T
    return np.tile(blk, (8, 1)).copy()


def _pack_stream(tok_idx_per_block, dcol_per_block, val_per_block, ch_per_block):
    """Build padded token stream + selector array for one core."""
    tot = sum(ch_per_block) * 128
    idx_s = np.zeros(tot, np.int64)
    pos_l = []
    col_l = []
    val_l = []
    base = 0
    for b in range(len(ch_per_block)):
        tok = tok_idx_per_block[b]
        n = len(tok)
        idx_s[base : base + n] = tok
        pos = base + np.arange(n)
        pos_l.append(pos)
        col_l.append(dcol_per_block[b])
        val_l.append(
            val_per_block[b] if val_per_block is not None else np.ones(n, np.float32)
        )
        base += ch_per_block[b] * 128
    pos = np.concatenate(pos_l) if pos_l else np.zeros(0, np.int64)
    col = np.concatenate(col_l).astype(np.int64) if col_l else np.zeros(0, np.int64)
    val = np.concatenate(val_l) if val_l else np.zeros(0, np.float32)
    return idx_s, pos, col, val


def _sel_array(pos, col, val, totc):
    """[128, totc*128] fp8 selector: S[pos%128, (pos//128)*128 + col] = val."""
    sel = np.zeros((128, totc * 128), np.float32)
    sel[pos % 128, (pos // 128) * 128 + col] = val
    return sel.astype(FP8)


def _pairs(c0, c1):
    """DoubleRow pairing measured slower on HW; emit singles."""
    return [(ci, 1) for ci in range(c0, c1)]


def _units(totc):
    """Split totc 128-token chunks into gather units of <=8 chunks."""
    out = []
    c0 = 0
    while c0 < totc:
        n = min(8, totc - c0)
        out.append((c0, n))
        c0 += n
    return out


def preprocess(inputs):
    """Host-side: shard, sort edges by dst, build index/selector tensors."""
    prot = np.asarray(inputs["protein_embedding"], np.float32)
    ipw = np.asarray(inputs["interpro_weight"], np.float32)
    W_esm = np.asarray(inputs["W_esm"], np.float32)
    b_esm = np.asarray(inputs["b_esm"], np.float32)
    bias1 = np.asarray(inputs["bias1"], np.float32)
    bias2 = np.asarray(inputs["bias2"], np.float32)
    w = np.asarray(inputs["w"], np.float32)
    W_upd = np.asarray(inputs["W_upd"], np.float32)
    b_upd = np.asarray(inputs["b_upd"], np.float32)
    W_out = np.asarray(inputs["W_out"], np.float32)
    b_out = np.asarray(inputs["b_out"], np.float32)
    self_w = np.asarray(inputs["self_w"], np.float32)
    ppi_w = np.asarray(inputs["ppi_w"], np.float32)
    node_in = np.asarray(inputs["inputs"], np.int64)
    ip_idx = np.asarray(inputs["interpro_idx"], np.int64)
    ip_off = np.asarray(inputs["interpro_off"], np.int64)
    src = np.asarray(inputs["src"], np.int64)
    dst = np.asarray(inputs["dst"], np.int64)
    target = np.asarray(inputs["target_id"], np.int64)

    ew = np.exp(w - w.max())
    sm = ew / ew.sum()

    bias_x1 = b_esm + bias1

    # --- edges: per (core, block) token lists sorted by dst ---
    order = np.argsort(dst, kind="stable")
    src_s, dst_s = src[order], dst[order]
    sw_s, pw_s = self_w[order], ppi_w[order]
    gblk = dst_s // 128
    blk_counts = np.bincount(gblk, minlength=N // 128)
    blk_starts = np.concatenate([[0], np.cumsum(blk_counts)])
    ch_e = np.zeros((NCORES, NBLK), np.int64)
    for c in range(NCORES):
        for b in range(NBLK):
            ch_e[c, b] = -(-blk_counts[c * NBLK + b] // 128)
    CH_E = ch_e.max(axis=0)
    TOTC_E = int(CH_E.sum())

    # --- bags ---
    bag_sizes = ip_off[1:] - ip_off[:-1]
    ch_b = np.zeros((NCORES, NBLK), np.int64)
    for c in range(NCORES):
        for b in range(NBLK):
            n0 = c * NS + b * 128
            cnt = int(ip_off[n0 + 128] - ip_off[n0])
            ch_b[c, b] = max(1, -(-cnt // 128))
    CH_B = ch_b.max(axis=0)
    TOTC_B = int(CH_B.sum())

    meta = dict(
        sm0=float(sm[0]),
        sm1=float(sm[1]),
        CH_E=[int(x) for x in CH_E],
        CH_B=[int(x) for x in CH_B],
        has_bias_x1=bool(np.any(bias_x1 != 0)),
        has_bias_x2=bool(np.any(bias2 != 0)),
        has_bias_upd=bool(np.any(b_upd != 0)),
        has_bias_out=bool(np.any(b_out != 0)),
    )

    W_esmT = np.ascontiguousarray(
        W_esm.T.reshape(KE, 128, D).transpose(1, 0, 2)
    ).astype(BF16)  # [128, KE, D]
    W_updT = np.ascontiguousarray(
        W_upd.transpose(0, 2, 1).reshape(G, KU, 128, D).transpose(0, 2, 1, 3)
    ).astype(BF16)  # [G, 128, KU, D]
    W_outP = np.zeros((18 * 128, LP), np.float32)
    W_outP[: D + D_ESM, :L] = W_out.T
    W_outT = np.ascontiguousarray(
        W_outP.reshape(18, 128, LP).transpose(1, 0, 2)
    ).astype(BF16)  # [128, 18, LP]
    # row-oriented biases (free-dim broadcast via ones-matmul)
    cbias = np.zeros((1, 128 + 4 * D), np.float32)
    cbias[0, :128] = 1.0
    cbias[0, 128 : 128 + D] = bias_x1
    cbias[0, 128 + D : 128 + 2 * D] = bias2
    cbias[0, 128 + 2 * D : 128 + 3 * D] = b_upd[0]
    cbias[0, 128 + 3 * D : 128 + 4 * D] = b_upd[1]
    # partition-oriented biases for transposed outputs (scalar bias add)
    cbT = np.zeros((128, 8 + NLS), np.float32)
    cbT[:, 0:8] = b_upd[1].reshape(8, 128).T
    cbT[:, 8 : 8 + NLS] = np.pad(b_out, (0, LP - L)).reshape(NLS, 128).T
    shared = dict(
        prot=prot.astype(BF16),
        ipw=(ipw * S_IPW).astype(FP8),
        W_esmT=W_esmT,
        W_updT=W_updT,
        W_outT=W_outT,
        cbias=cbias.astype(BF16),
        cbT=cbT.astype(np.float32),
    )

    in_maps = []
    for c in range(NCORES):
        esm_idx = _wrap_idx(node_in[c * NS : (c + 1) * NS], NS)
        tgt_idx = _wrap_idx(target[c * NS : (c + 1) * NS], NS)

        tokb, colb, valsb, valpb = [], [], [], []
        for b in range(NBLK):
            s0, s1 = blk_starts[c * NBLK + b], blk_starts[c * NBLK + b + 1]
            tokb.append(src_s[s0:s1])
            colb.append(dst_s[s0:s1] - (c * NS + b * 128))
            valsb.append(sw_s[s0:s1])
            valpb.append(pw_s[s0:s1])
        eidx, epos, ecol, esv = _pack_stream(tokb, colb, valsb, list(CH_E))
        _, _, _, epv = _pack_stream(tokb, colb, valpb, list(CH_E))
        sel_self = _sel_array(epos, ecol, esv, TOTC_E)
        sel_ppi = _sel_array(epos, ecol, epv, TOTC_E)

        tokb, colb = [], []
        for b in range(NBLK):
            n0 = c * NS + b * 128
            i0, i1 = int(ip_off[n0]), int(ip_off[n0 + 128])
            tokb.append(ip_idx[i0:i1])
            colb.append(
                np.repeat(
                    np.arange(128), bag_sizes[n0 : n0 + 128].astype(np.int64)
                )
            )
        bidx, bpos, bcol, bval = _pack_stream(tokb, colb, None, list(CH_B))
        sel_bag = _sel_array(bpos, bcol, bval, TOTC_B)

        m = dict(shared)
        m.update(
            esm_idx=esm_idx,
            tgt_idx=tgt_idx,
            e_idx=_wrap_idx(eidx, TOTC_E * 128),
            b_idx=_wrap_idx(bidx, TOTC_B * 128),
            sel_self=sel_self,
            sel_ppi=sel_ppi,
            sel_bag=sel_bag,
        )
        in_maps.append(m)
    return meta, in_maps


def build(meta):
    CH_E = meta["CH_E"]
    CH_B = meta["CH_B"]
    TOTC_E = sum(CH_E)
    TOTC_B = sum(CH_B)
    sm0, sm1 = meta["sm0"], meta["sm1"]
    bf = mybir.dt.bfloat16
    f32 = mybir.dt.float32
    f8 = mybir.dt.float8e4
    i16 = mybir.dt.int16
    RELU = mybir.ActivationFunctionType.Relu
    COPY = mybir.ActivationFunctionType.Copy

    nc = bacc.Bacc("TRN2", target_bir_lowering=False, debug=False,
                   num_devices=NCORES)
    t_prot = nc.dram_tensor("prot", [P, D_ESM], bf, kind="ExternalInput")
    t_ipw = nc.dram_tensor("ipw", [IP, D], f8, kind="ExternalInput")
    t_Wesm = nc.dram_tensor("W_esmT", [128, KE, D], bf, kind="ExternalInput")
    t_Wupd = nc.dram_tensor("W_updT", [G, 128, KU, D], bf, kind="ExternalInput")
    t_Wout = nc.dram_tensor("W_outT", [128, 18, LP], bf, kind="ExternalInput")
    t_cbias = nc.dram_tensor("cbias", [1, 128 + 4 * D], bf, kind="ExternalInput")
    t_cbT = nc.dram_tensor("cbT", [128, 8 + NLS], f32, kind="ExternalInput")
    t_esmi = nc.dram_tensor("esm_idx", [128, NS // 16], i16, kind="ExternalInput")
    t_tgti = nc.dram_tensor("tgt_idx", [128, NS // 16], i16, kind="ExternalInput")
    t_eidx = nc.dram_tensor("e_idx", [128, TOTC_E * 8], i16, kind="ExternalInput")
    t_bidx = nc.dram_tensor("b_idx", [128, TOTC_B * 8], i16, kind="ExternalInput")
    t_selfS = nc.dram_tensor("sel_self", [128, TOTC_E * 128], f8, kind="ExternalInput")
    t_ppiS = nc.dram_tensor("sel_ppi", [128, TOTC_E * 128], f8, kind="ExternalInput")
    t_bagS = nc.dram_tensor("sel_bag", [128, TOTC_B * 128], f8, kind="ExternalInput")

    if PHASES >= 3:
        t_out = nc.dram_tensor("out", [LP, NS], f32, kind="ExternalOutput")
    elif PHASES == 2:
        t_out = nc.dram_tensor("out", [D, NS], f32, kind="ExternalOutput")
    else:
        t_out = nc.dram_tensor("out", [NS, D], f32, kind="ExternalOutput")

    def blk_ranges(CH):
        r, c0 = [], 0
        for b in range(NBLK):
            r.append((c0, c0 + CH[b]))
            c0 += CH[b]
        return r

    BR_E = blk_ranges(CH_E)
    BR_B = blk_ranges(CH_B)
    U_E = _units(TOTC_E)
    U_B = _units(TOTC_B)

    with tile.TileContext(nc) as tc:
        with (
            tc.tile_pool(name="static", bufs=1) as stat,
            tc.tile_pool(name="dram", bufs=1, space="DRAM") as dram,
        ):
            any_bias = (meta["has_bias_x1"] or meta["has_bias_x2"]
                        or meta["has_bias_upd"] or meta["has_bias_out"])
            if any_bias:
                cb = stat.tile([1, 128 + 4 * D], bf)
                nc.sync.dma_start(cb[:], t_cbias[:])
                cbT_s = stat.tile([128, 8 + NLS], f32)
                nc.sync.dma_start(cbT_s[:], t_cbT[:])
                ones = cb[0:1, 0:128]
            else:
                cb = None
                cbT_s = None
                ones = None
            eidx_s = stat.tile([128, TOTC_E * 8], i16)
            nc.sync.dma_start(eidx_s[:], t_eidx[:])

            # DRAM intermediates
            hb = [None, None]
            hf = [None, None]
            for li in range(2):
                hf[li] = dram.tile([N, D], f8, tag=f"hf{li}", name=f"hf{li}",
                                   addr_space="Shared")
                hb[li] = dram.tile([NS, D], f8, tag=f"hb{li}", name=f"hb{li}")
            partial = dram.tile([LP, NS], bf, name="partial")

            def emit_allgather(li):
                nc.gpsimd.collective_compute(
                    "AllGather", mybir.AluOpType.bypass,
                    replica_groups=[list(range(NCORES))],
                    ins=[hb[li][:].opt()],
                    outs=[hf[li][:].opt()],
                )

            def emit_lsub(wp, fp, pse, wt_cache, ls):
                """Emit one esm-partial L-sub (two node-halves)."""
                lg = (ls // 4) * 4
                if lg not in wt_cache:
                    wt = wp.tile([128, KE, 512], bf, tag="we")
                    nc.sync.dma_start(
                        wt[:], t_Wout[:, 8:18, lg * 128 : (lg + 4) * 128]
                    )
                    wt_cache.clear()
                    wt_cache[lg] = wt
                wt = wt_cache[lg]
                li = ls - lg
                for half in range(2):
                    ps = pse.tile([128, D], f32, tag="pse")
                    for kk in range(KE):
                        lhsT = wt[:, kk, li * 128 : (li + 1) * 128]
                        for ng in range(2):
                            nc.tensor.matmul(
                                ps[:, ng * 512 : (ng + 1) * 512],
                                lhsT,
                                esmtT[:, kk,
                                      half * D + ng * 512
                                      : half * D + (ng + 1) * 512],
                                start=(kk == 0), stop=(kk == KE - 1),
                            )
                    pt = fp.tile([128, D], bf, tag="pt")
                    nc.vector.tensor_copy(pt[:], ps[:])
                    nc.sync.dma_start(
                        partial[ls * 128 : (ls + 1) * 128,
                                half * D : (half + 1) * D],
                        pt[:],
                    )

            def esm_final_part(ls0, ls1):
                """out^T partial for L-subs [ls0, ls1): esm k-chunks only."""
                with (
                    tc.tile_pool(name=f"we{ls0}", bufs=2) as wp,
                    tc.tile_pool(name=f"pf{ls0}", bufs=3) as fp,
                    tc.tile_pool(name=f"psE{ls0}", bufs=2, space="PSUM") as pse,
                ):
                    wt_cache = {}
                    for ls in range(ls0, ls1):
                        emit_lsub(wp, fp, pse, wt_cache, ls)

            # esmtT lives from phase A through esm-final part 2, then its
            # SBUF is released for layer 1 (manual pool enter/exit keeps
            # the block flat).
            esmf_ctx = tc.tile_pool(name="esmf", bufs=1)
            esmf_p = esmf_ctx.__enter__()
            esmtT = esmf_p.tile([128, KE, NS], bf, name="esmtT")

            # ---------------- Phase A: x1 + x2 -> h0 (fp8, x S_H[0]) -------
            N_INL = 13  # esm-final L-subs interleaved into phase A
            with (
                tc.tile_pool(name="esmT", bufs=1) as esmT_p,
                tc.tile_pool(name="tgg", bufs=2) as tgg_p,
                tc.tile_pool(name="msg", bufs=3) as msg_p,
                tc.tile_pool(name="sel", bufs=3) as sel_p,
                tc.tile_pool(name="hmix", bufs=3) as hmix_p,
                tc.tile_pool(name="weA", bufs=2) as wpA,
                tc.tile_pool(name="pfA", bufs=3) as fpA,
                tc.tile_pool(name="psA", bufs=2, space="PSUM") as psA,
                tc.tile_pool(name="psEA", bufs=2, space="PSUM") as pseA,
            ):
                wtcA = {}
                lsub_done = 0
                Wesm_s = esmT_p.tile([128, KE, D], bf)
                nc.sync.dma_start(Wesm_s[:], t_Wesm[:])
                esmi_s = esmT_p.tile([128, NS // 16], i16)
                nc.sync.dma_start(esmi_s[:], t_esmi[:])
                tgti_s = esmT_p.tile([128, NS // 16], i16)
                nc.sync.dma_start(tgti_s[:], t_tgti[:])
                bidx_s = esmT_p.tile([128, TOTC_B * 8], i16)
                nc.sync.dma_start(bidx_s[:], t_bidx[:])
                esmT = []
                for u in range(NBLK):
                    tl = esmT_p.tile([128, KE, 128], bf, tag="esmT", bufs=NBLK)
                    nc.gpsimd.dma_gather(
                        tl[:], t_prot[:], esmi_s[:, u * 8 : (u + 1) * 8],
                        128, 128, D_ESM, transpose=True,
                    )
                    esmT.append(tl)

                # target-protein gathers for the esm-final trick (early so
                # interleaved esm-final L-subs can start mid-phase)
                for u in range(NBLK):
                    tg = tgg_p.tile([128, KE, 128], bf, tag="tg")
                    nc.gpsimd.dma_gather(
                        tg[:], t_prot[:], tgti_s[:, u * 8 : (u + 1) * 8],
                        128, 128, D_ESM, transpose=True,
                    )
                    nc.vector.tensor_copy(
                        esmtT[:, :, u * 128 : (u + 1) * 128], tg[:]
                    )

                # bag gathers (issued early; consumed per-chunk below)
                bmsg = {}
                bsel = {}
                for ui, (c0, nch) in enumerate(U_B):
                    mt = msg_p.tile([128, 8, D], f8, tag="msg")
                    nc.gpsimd.dma_gather(
                        mt[:, 0:nch, :], t_ipw[:],
                        bidx_s[:, c0 * 8 : (c0 + nch) * 8],
                        nch * 128, nch * 128, D, single_packet=False,
                    )
                    st = sel_p.tile([128, 8, 128], f8, tag="sel")
                    nc.sync.dma_start(
                        st[:, 0:nch, :],
                        t_bagS[:, c0 * 128 : (c0 + nch) * 128],
                    )
                    bmsg[ui] = mt
                    bsel[ui] = st

                for nt in range(NBLK):
                    # x1 psum
                    ps1 = psA.tile([128, D], f32, tag="ps")
                    for jj in range(KE):
                        for b in range(2):
                            nc.tensor.matmul(
                                ps1[:, b * 512 : (b + 1) * 512],
                                esmT[nt][:, jj, :],
                                Wesm_s[:, jj, b * 512 : (b + 1) * 512],
                                start=(jj == 0),
                                stop=(jj == KE - 1 and not meta["has_bias_x1"]),
                            )
                    if meta["has_bias_x1"]:
                        for b in range(2):
                            nc.tensor.matmul(
                                ps1[:, b * 512 : (b + 1) * 512], ones,
                                cb[0:1, 128 + b * 512 : 128 + (b + 1) * 512],
                                start=False, stop=True,
                            )
                    # x2 psum: selector matmuls over this block's chunks
                    ps2 = psA.tile([128, D], f32, tag="ps")
                    c0, c1 = BR_B[nt]
                    prs = _pairs(c0, c1)
                    for pi, (ci, n) in enumerate(prs):
                        mt, st = bmsg[ci // 8], bsel[ci // 8]
                        j = ci % 8
                        first = pi == 0
                        last = (pi == len(prs) - 1
                                and not meta["has_bias_x2"])
                        for b in range(2):
                            if n == 2:
                                nc.tensor.matmul(
                                    ps2[:, b * 512 : (b + 1) * 512],
                                    st[:, j : j + 2, :],
                                    mt[:, j : j + 2, b * 512 : (b + 1) * 512],
                                    start=first, stop=last,
                                    perf_mode=mybir.MatmulPerfMode.DoubleRow,
                                )
                            else:
                                nc.tensor.matmul(
                                    ps2[:, b * 512 : (b + 1) * 512],
                                    st[:, j, :],
                                    mt[:, j, b * 512 : (b + 1) * 512],
                                    start=first, stop=last,
                                )
                    if meta["has_bias_x2"]:
                        for b in range(2):
                            nc.tensor.matmul(
                                ps2[:, b * 512 : (b + 1) * 512], ones,
                                cb[0:1, 128 + D + b * 512 : 128 + D + (b + 1) * 512],
                                start=False, stop=True,
                            )
                    # mix (scaled by S_H[0]): h8 = sm0*S*relu(x1) + sm1*S*relu(x2)
                    m1 = hmix_p.tile([128, D], bf, tag="m1")
                    m2 = hmix_p.tile([128, D], bf, tag="m2")
                    h8 = hmix_p.tile([128, D], f8, tag="h8")
                    nc.scalar.activation(m1[:], ps1[:], RELU, scale=sm0 * S_H[0])
                    nc.scalar.activation(
                        m2[:], ps2[:], RELU, scale=sm1 * S_H[0] / S_IPW
                    )
                    nc.vector.tensor_add(h8[:], m1[:], m2[:])
                    if PHASES <= 1:
                        hf32 = hmix_p.tile([128, D], f32, tag="hf32")
                        nc.scalar.activation(
                            hf32[:], h8[:], COPY, scale=1.0 / S_H[0]
                        )
                        nc.sync.dma_start(
                            t_out[nt * 128 : (nt + 1) * 128, :], hf32[:]
                        )
                    nc.sync.dma_start(
                        hb[0][nt * 128 : (nt + 1) * 128, :], h8[:]
                    )
                    if nt >= 2:
                        want = (nt - 1) * N_INL // (NBLK - 2)
                        while lsub_done < min(want, N_INL):
                            emit_lsub(wpA, fpA, pseA, wtcA, lsub_done)
                            lsub_done += 1
                    if nt == NBLK - 1:
                        emit_allgather(0)

            if PHASES <= 1:
                esmf_ctx.__exit__(None, None, None)
                return nc

            # esm-final part 1 fills the h0 AllGather wall
            esm_final_part(13, 23)

            # ---------------- GNN layers ----------------
            def segsum_layer(layer, catT_all, upd_fn):
                mbufs = 5 if layer == 0 else 4
                with (
                    tc.tile_pool(name=f"msgL{layer}", bufs=mbufs) as msg_p,
                    tc.tile_pool(name=f"selL{layer}", bufs=3) as sel_p,
                    tc.tile_pool(name=f"catL{layer}", bufs=2) as cat_p,
                    tc.tile_pool(name=f"psS{layer}", bufs=1, space="PSUM") as ps_p,
                ):
                    emsg, esel_s, esel_p = {}, {}, {}
                    for ui, (c0, nch) in enumerate(U_E):
                        mt = msg_p.tile([128, 8, D], f8, tag="msg")
                        nc.gpsimd.dma_gather(
                            mt[:, 0:nch, :], hf[layer][:],
                            eidx_s[:, c0 * 8 : (c0 + nch) * 8],
                            nch * 128, nch * 128, D, single_packet=False,
                        )
                        s1 = sel_p.tile([128, 8, 128], f8, tag="sself")
                        nc.sync.dma_start(
                            s1[:, 0:nch, :],
                            t_selfS[:, c0 * 128 : (c0 + nch) * 128],
                        )
                        s2 = sel_p.tile([128, 8, 128], f8, tag="sppi")
                        nc.sync.dma_start(
                            s2[:, 0:nch, :],
                            t_ppiS[:, c0 * 128 : (c0 + nch) * 128],
                        )
                        emsg[ui], esel_s[ui], esel_p[ui] = mt, s1, s2

                    inv = 1.0 / S_H[layer]
                    for blk in range(NBLK):
                        # cat psum: cols [0:D]=ppi, [D:2D]=res
                        cps = ps_p.tile([128, 2 * D], f32, tag="cps")
                        c0, c1 = BR_E[blk]
                        prs = _pairs(c0, c1)
                        for pi, (ci, n) in enumerate(prs):
                            mt = emsg[ci // 8]
                            j = ci % 8
                            first = pi == 0
                            last = pi == len(prs) - 1
                            sp = esel_p[ci // 8]
                            ss = esel_s[ci // 8]
                            for off, sel in ((0, sp), (D, ss)):
                                for b in range(2):
                                    if n == 2:
                                        nc.tensor.matmul(
                                            cps[:, off + b * 512
                                                : off + (b + 1) * 512],
                                            sel[:, j : j + 2, :],
                                            mt[:, j : j + 2,
                                               b * 512 : (b + 1) * 512],
                                            start=first, stop=last,
                                            perf_mode=(
                                                mybir.MatmulPerfMode.DoubleRow
                                            ),
                                        )
                                    else:
                                        nc.tensor.matmul(
                                            cps[:, off + b * 512
                                                : off + (b + 1) * 512],
                                            sel[:, j, :],
                                            mt[:, j, b * 512 : (b + 1) * 512],
                                            start=first, stop=last,
                                        )
                        catt = cat_p.tile([128, 2 * D], bf, tag="cat")
                        nc.scalar.activation(catt[:], cps[:], COPY, scale=inv)
                        nc.sync.dma_start_transpose(
                            catT_all[:, :, blk * 128 : (blk + 1) * 128], catt[:]
                        )
                        # interleave the previous block's update matmuls so
                        # the PE fills gather stalls with useful work
                        if blk >= 1:
                            upd_fn(blk - 1)
                    upd_fn(NBLK - 1)

            # ---- layer 0 (update interleaved into segsum) ----
            with (
                tc.tile_pool(name="l0", bufs=1) as l0_p,
                tc.tile_pool(name="hn0", bufs=3) as hn_p,
                tc.tile_pool(name="psU0", bufs=2, space="PSUM") as psU,
            ):
                catT0 = l0_p.tile([128, KU, NS], bf)
                Wu0 = l0_p.tile([128, KU, D], bf)
                nc.sync.dma_start(Wu0[:], t_Wupd[0])

                def upd0(blk):
                    psu = psU.tile([128, D], f32, tag="psu", name="psu")
                    for kk in range(KU):
                        for b in range(2):
                            nc.tensor.matmul(
                                psu[:, b * 512 : (b + 1) * 512],
                                catT0[:, kk, blk * 128 : (blk + 1) * 128],
                                Wu0[:, kk, b * 512 : (b + 1) * 512],
                                start=(kk == 0),
                                stop=(kk == KU - 1
                                      and not meta["has_bias_upd"]),
                            )
                    if meta["has_bias_upd"]:
                        for b in range(2):
                            nc.tensor.matmul(
                                psu[:, b * 512 : (b + 1) * 512], ones,
                                cb[0:1, 128 + 2 * D + b * 512
                                   : 128 + 2 * D + (b + 1) * 512],
                                start=False, stop=True,
                            )
                    ht = hn_p.tile([128, D], f8, tag="h", name="ht")
                    nc.scalar.activation(ht[:], psu[:], RELU, scale=S_H[1])
                    nc.sync.dma_start(
                        hb[1][blk * 128 : (blk + 1) * 128, :], ht[:]
                    )
                    if blk == NBLK - 1:
                        emit_allgather(1)

                segsum_layer(0, catT0, upd0)

            # esm-final part 2 fills the h1 AllGather wall
            esm_final_part(23, NLS)
            esmf_ctx.__exit__(None, None, None)

            # ---- layer 1 (h2^T stays in SBUF) + final ----
            with tc.tile_pool(name="late", bufs=1) as late_p:
                h2T = late_p.tile([128, D // 128, NS], bf)
                with (
                    tc.tile_pool(name="l1", bufs=1) as l1_p,
                    tc.tile_pool(name="hn1", bufs=3) as hn1_p,
                    tc.tile_pool(name="psU1", bufs=2, space="PSUM") as psU,
                ):
                    catT1 = l1_p.tile([128, KU, NS], bf)
                    Wu1 = l1_p.tile([128, KU, D], bf)
                    nc.sync.dma_start(Wu1[:], t_Wupd[1])

                    def upd1(blk):
                        psu = psU.tile([128, D], f32, tag="psu", name="psu1")
                        for kk in range(KU):
                            for b in range(2):
                                nc.tensor.matmul(
                                    psu[:, b * 512 : (b + 1) * 512],
                                    catT1[:, kk, blk * 128 : (blk + 1) * 128],
                                    Wu1[:, kk, b * 512 : (b + 1) * 512],
                                    start=(kk == 0),
                                    stop=(kk == KU - 1
                                          and not meta["has_bias_upd"]),
                                )
                        if meta["has_bias_upd"]:
                            for b in range(2):
                                nc.tensor.matmul(
                                    psu[:, b * 512 : (b + 1) * 512], ones,
                                    cb[0:1, 128 + 3 * D + b * 512
                                       : 128 + 3 * D + (b + 1) * 512],
                                    start=False, stop=True,
                                )
                        ht = hn1_p.tile([128, D], bf, tag="h", name="ht1")
                        nc.scalar.activation(ht[:], psu[:], RELU)
                        nc.sync.dma_start_transpose(
                            h2T[:, :, blk * 128 : (blk + 1) * 128], ht[:]
                        )

                    segsum_layer(1, catT1, upd1)

                if PHASES == 2:
                    with tc.tile_pool(name="dbg", bufs=4) as dbg_p:
                        for ds in range(D // 128):
                            ff = dbg_p.tile([128, NS], f32, tag="ff")
                            nc.vector.tensor_copy(ff[:], h2T[:, ds, :])
                            nc.sync.dma_start(
                                t_out[ds * 128 : (ds + 1) * 128, :], ff[:])
                    return nc

                # ---- Final: out^T = W_h2-chunks^T @ h2T + partial ----
                with (
                    tc.tile_pool(name="wout", bufs=2) as wout_p,
                    tc.tile_pool(name="fin", bufs=3) as fin_p,
                    tc.tile_pool(name="psF", bufs=2, space="PSUM") as psF,
                ):
                    for lg in range(0, NLS, 4):
                        wt = wout_p.tile([128, 8, 512], bf, tag="wo")
                        nc.sync.dma_start(
                            wt[:], t_Wout[:, 0:8, lg * 128 : (lg + 4) * 128]
                        )
                        for li in range(4):
                            ls = lg + li
                            ps = psF.tile([128, NS], f32, tag="psf")
                            for kk in range(8):
                                lhsT = wt[:, kk, li * 128 : (li + 1) * 128]
                                for ng in range(4):
                                    nc.tensor.matmul(
                                        ps[:, ng * 512 : (ng + 1) * 512],
                                        lhsT,
                                        h2T[:, kk, ng * 512 : (ng + 1) * 512],
                                        start=(kk == 0), stop=(kk == 7),
                                    )
                            pt = fin_p.tile([128, NS], bf, tag="pt")
                            nc.sync.dma_start(
                                pt[:], partial[ls * 128 : (ls + 1) * 128, :]
                            )
                            ot = fin_p.tile([128, NS], f32, tag="o")
                            nc.vector.tensor_add(ot[:], ps[:], pt[:])
                            if meta["has_bias_out"]:
                                nc.vector.tensor_scalar_add(
                                    ot[:], ot[:], cbT_s[:, 8 + ls : 8 + ls + 1]
                                )
                            nc.sync.dma_start(
                                t_out[ls * 128 : (ls + 1) * 128, :], ot[:]
                            )
    return nc


def kernel(**inputs):
    meta, in_maps = preprocess(inputs)
    nc = build(meta)
    nc.compile()
    res = bass_utils.run_bass_kernel_spmd(
        nc, in_maps, core_ids=list(range(NCORES)), trace=TRACE
    )
    kernel.last_exec_ns = res.exec_time_ns
    if PHASES >= 3:
        out = np.concatenate(
            [res.results[c]["out"][:L, :].T for c in range(NCORES)], axis=0
        )
        out = np.ascontiguousarray(out)
    elif PHASES == 2:
        kernel.per_core = [
            np.ascontiguousarray(res.results[c]["out"].T) for c in range(NCORES)
        ]
        out = kernel.per_core[0]
    else:
        out = res.results[0]["out"]
        kernel.per_core = [res.results[c]["out"] for c in range(NCORES)]
    return out
